# revision 4
# baseline (speedup 1.0000x reference)
"""Haar-DWT downsampling + 1x1 conv + BN + ReLU fused Trainium2 kernel.

Math: the Haar DWT (J=1) followed by a 1x1 conv over the 4C subband
channels, inference BN, and ReLU is one linear op + bias + ReLU.  It
folds into a 2x2/stride-2 conv:

    z[o, i, j] = relu( sum_{c,di,dj} Weff[o, c, di, dj] * x[c, 2i+di, 2j+dj]
                       + bias_total[o] )

with Weff/bias_total computed on the host from (W, b, gamma, beta, mean,
var).  On-device this is, per output tile, 2 accumulating matmuls
(contraction K = 128 = (c, di), one per dj) + one scalar-engine
activation (bias + ReLU) reading PSUM.

Sharding: pure data-parallel over batch. B=16 -> 2 images per core on
8 cores. Each core reads only its x shard and writes only its z shard
(minimal HBM traffic: 33.5 MB in + 16.8 MB out per core).
"""

import numpy as np

import concourse.bass as bass
import concourse.bacc as bacc
import concourse.mybir as mybir
from concourse.tile import TileContext
from concourse.bass_utils import run_bass_kernel_spmd

BN_EPS = 1e-5

# Problem shape (hardcoded per harness contract)
B, C, H, W_IMG = 16, 64, 256, 256
COUT = 128
N_CORES = 8
B_LOCAL = B // N_CORES          # 2 images per core
HO, WO = H // 2, W_IMG // 2     # 128 x 128 output image

NROWS = 16                      # output rows per tile
N_ROW_BLOCKS = HO // NROWS      # 8
GROUPS = (NROWS * WO) // 512    # matmul free-dim groups of 512

F32 = mybir.dt.float32
F32R = mybir.dt.float32r


def _fold_weights(W, b, gamma, beta, mean, var):
    """Fold DWT + conv + BN into per-(di,dj) lhsT weights
    [4, 128(K), 128(M=o)] and a per-channel bias [COUT].

    Combo q = di*2 + dj.  K rows 0-63 and 64-127 hold the SAME c-indexed
    weights (duplicated): the kernel packs two K=64 matmuls into the PE
    array (partition halves 0/64), one per h-half of the input tile, and
    lhsT/rhs base partitions must match.
    """
    W = W.astype(np.float64)
    Wll, Wlh, Whl, Whh = W[:, :C], W[:, C:2 * C], W[:, 2 * C:3 * C], W[:, 3 * C:]
    s = (gamma.astype(np.float64) / np.sqrt(var.astype(np.float64) + BN_EPS))
    coef = {
        (0, 0): 0.5 * (Wll + Wlh + Whl + Whh),
        (0, 1): 0.5 * (Wll + Wlh - Whl - Whh),
        (1, 0): 0.5 * (Wll - Wlh + Whl - Whh),
        (1, 1): 0.5 * (Wll - Wlh - Whl + Whh),
    }
    bias_total = (b.astype(np.float64) * s + beta.astype(np.float64)
                  - mean.astype(np.float64) * s)
    lhsT = np.zeros((4, 128, COUT), dtype=np.float64)
    for di in range(2):
        for dj in range(2):
            wq = (coef[(di, dj)] * s[:, None]).T   # [c, o]
            lhsT[di * 2 + dj, 0:C, :] = wq
            lhsT[di * 2 + dj, C:2 * C, :] = wq
    return lhsT.astype(np.float32), bias_total.astype(np.float32)


def build_nc(b_local=B_LOCAL, n_row_blocks=4, use_f32r=True,
             run_bacc_compile=True):
    """n_row_blocks: 32-output-row blocks per image (full image = 4)."""
    nc = bacc.Bacc(None)
    mm_dt = F32R if use_f32r else F32
    x = nc.dram_tensor("x", [b_local, C, H, W_IMG], mm_dt, kind="ExternalInput")
    w_lhsT = nc.dram_tensor("w_lhsT", [4, 128, COUT], mm_dt, kind="ExternalInput")
    bias = nc.dram_tensor("bias", [COUT, 1], F32, kind="ExternalInput")
    z = nc.dram_tensor("z", [b_local, COUT, HO, WO], F32, kind="ExternalOutput")

    with TileContext(nc) as tc:
        with (
            tc.tile_pool(name="consts", bufs=1) as cpool,
            tc.tile_pool(name="xin", bufs=4) as xpool,
            tc.tile_pool(name="psum", bufs=2, space="PSUM") as ppool,
            tc.tile_pool(name="zout", bufs=3) as zpool,
        ):
            w_sb = []
            for q in range(4):
                wt = cpool.tile([128, COUT], mm_dt, name=f"w{q}_sb")
                nc.sync.dma_start(out=wt[:], in_=w_lhsT[q])
                w_sb.append(wt)
            bias_sb = cpool.tile([COUT, 1], F32)
            nc.sync.dma_start(out=bias_sb[:], in_=bias[:])

            zv = z.rearrange("b o (hb i2) j -> b o hb (i2 j)", hb=HO // 16)

            for bi in range(b_local):
                for tb in range(n_row_blocks):
                    # 64 input rows -> 32 output rows; partition =
                    # (hhalf, c): each partition holds 32 contiguous
                    # input rows (32 KB) -> one fully-contiguous 4 MB DMA.
                    xt = xpool.tile([128, 32 * W_IMG], mm_dt)
                    # One DMA per h-half: DRAM-side outer dim = c (64) so
                    # descriptors spray across all 16 SDMA engines.  A single
                    # (hh c) DMA has outer dim hh=2 -> only 2 engines carry
                    # the whole 4 MiB (measured 26 GB/s x 2 = whole-kernel
                    # bottleneck).
                    for hh in range(2):
                        src_h = x[bi, :, 64 * tb + 32 * hh:
                                 64 * tb + 32 * (hh + 1), :].rearrange(
                            "c hl w -> c (hl w)"
                        )
                        nc.sync.dma_start(
                            out=xt[64 * hh:64 * (hh + 1), :], in_=src_h)
                    # free f = il*512 + di*256 + j*2 + dj  (il<16 per half)
                    xv = xt.rearrange(
                        "p (il di j dj) -> p di dj il j", di=2, j=WO, dj=2
                    )
                    for pt in range(2):   # two psum tiles per block
                        ps = ppool.tile([COUT, 2048], F32)
                        # region h*1024 + gg*512 <- output rows
                        # (32tb + 16h + 8pt + 4gg + 0..3)
                        for q in range(4):
                            di, dj = q // 2, q % 2
                            for h in range(2):
                                lw = w_sb[q][h * C:(h + 1) * C, :]
                                for gg in range(2):
                                    il0 = 8 * pt + 4 * gg
                                    nc.tensor.matmul(
                                        ps[:, h * 1024 + gg * 512:
                                           h * 1024 + gg * 512 + 512],
                                        lhsT=lw,
                                        rhs=xv[h * C:(h + 1) * C, di, dj,
                                               il0:il0 + 4, :],
                                        start=(q == 0),
                                        stop=(q == 3),
                                    )
                        zt = zpool.tile([COUT, 2048], F32)
                        # bias + ReLU in one DVE pass: max(ps + bias, 0)
                        nc.vector.tensor_scalar(
                            zt[:], ps[:], bias_sb[:, 0:1], 0.0,
                            mybir.AluOpType.add, mybir.AluOpType.max,
                        )
                        # rows (32tb+8pt..+8) and (32tb+16+8pt..+8):
                        # hb in {2tb, 2tb+1}, f window 1024*pt..+1024.
                        # Issued on the ACT HWDGE ring so a store waiting on
                        # DVE never blocks the next x load on the SP ring.
                        nc.scalar.dma_start(
                            out=zv[bi, :, 2 * tb:2 * tb + 2,
                                   1024 * pt:1024 * pt + 1024],
                            in_=zt.rearrange("o (hh f) -> o hh f", hh=2),
                        )
    if run_bacc_compile:
        nc.compile()
    return nc


_NC_CACHE = {}


def _get_nc():
    if "nc" not in _NC_CACHE:
        _NC_CACHE["nc"] = build_nc()
    return _NC_CACHE["nc"]


def kernel(x, W, b, gamma, beta, mean, var, _trace=False):
    x = np.ascontiguousarray(np.asarray(x, dtype=np.float32))
    lhsT, bias_total = _fold_weights(
        np.asarray(W), np.asarray(b), np.asarray(gamma),
        np.asarray(beta), np.asarray(mean), np.asarray(var),
    )
    bias_col = np.ascontiguousarray(bias_total.reshape(COUT, 1))

    nc = _get_nc()
    in_maps = []
    for core in range(N_CORES):
        xs = np.ascontiguousarray(x[core * B_LOCAL:(core + 1) * B_LOCAL])
        in_maps.append({"x": xs, "w_lhsT": lhsT, "bias": bias_col})

    res = run_bass_kernel_spmd(
        nc, in_maps, list(range(N_CORES)), trace=_trace
    )
    out = np.concatenate([res.results[i]["z"] for i in range(N_CORES)], axis=0)
    if _trace:
        return out, res
    return out



# revision 5
# speedup vs baseline: 1.5313x; 1.5313x over previous
"""Haar-DWT downsampling + 1x1 conv + BN + ReLU fused Trainium2 kernel.

Math: the Haar DWT (J=1) followed by a 1x1 conv over the 4C subband
channels, inference BN, and ReLU is one linear op + bias + ReLU.  It
folds into a 2x2/stride-2 conv:

    z[o, i, j] = relu( sum_{c,di,dj} Weff[o, c, di, dj] * x[c, 2i+di, 2j+dj]
                       + bias_total[o] )

with Weff/bias_total computed on the host from (W, b, gamma, beta, mean,
var).  On-device this is, per output tile, 4 accumulating matmuls
(contraction K = 64 c's per (di,dj) tap) + one DVE pass (bias + ReLU).

Precision: x and the folded weights are cast to bf16 on the host and z
is produced in bf16 (PSUM accumulation stays f32) — halves HBM traffic,
which is the roofline here; measured rel err ~2e-3 vs the 2e-2 gate.

Layout: x is pre-tiled on the host to [b, tb, hh, c, hl, w] so each
(bi, tb, hh) load is one fully-contiguous 2 MiB DMA whose DRAM-side
outer dim is c=64 -> descriptors spray across all 16 SDMA engines
(32 KiB each).  z is written as [b, hb, o, 16*128] tiles (contiguous
per store, 4 KiB descriptors, outer dim o=128) and un-tiled on the
host.  Loads issue on the SP HWDGE ring, stores on the ACT ring, so a
store waiting on compute never blocks a load.

Sharding: pure data-parallel over batch. B=16 -> 2 images per core on
8 cores.
"""

import numpy as np
import ml_dtypes

import concourse.bass as bass
import concourse.bacc as bacc
import concourse.mybir as mybir
from concourse.tile import TileContext
from concourse.bass_utils import run_bass_kernel_spmd

BN_EPS = 1e-5

# Problem shape (hardcoded per harness contract)
B, C, H, W_IMG = 16, 64, 256, 256
COUT = 128
N_CORES = 8
B_LOCAL = B // N_CORES          # 2 images per core
HO, WO = H // 2, W_IMG // 2     # 128 x 128 output image

TB = 2                          # row blocks per image (128 input rows each)
ROWS_IN = H // TB               # 128 input rows per block
HL = ROWS_IN // 2               # 64 input rows per (block, hh) partition half
IL = HL // 2                    # 32 output rows per partition half
HB = 8                          # 16-output-row store blocks per image

F32 = mybir.dt.float32
BF16 = mybir.dt.bfloat16


def _fold_weights(W, b, gamma, beta, mean, var):
    """Fold DWT + conv + BN into per-(di,dj) lhsT weights
    [4, 128(K), 128(M=o)] (bf16) and a per-channel f32 bias [COUT, 1].

    K rows 0-63 and 64-127 hold the SAME c-indexed weights (duplicated):
    the kernel runs K=64 matmuls against either partition half of the x
    tile (hh = which 64-row half), and lhsT/rhs base partitions must
    match.
    """
    W = W.astype(np.float64)
    Wll, Wlh, Whl, Whh = W[:, :C], W[:, C:2 * C], W[:, 2 * C:3 * C], W[:, 3 * C:]
    s = (gamma.astype(np.float64) / np.sqrt(var.astype(np.float64) + BN_EPS))
    coef = {
        (0, 0): 0.5 * (Wll + Wlh + Whl + Whh),
        (0, 1): 0.5 * (Wll + Wlh - Whl - Whh),
        (1, 0): 0.5 * (Wll - Wlh + Whl - Whh),
        (1, 1): 0.5 * (Wll - Wlh - Whl + Whh),
    }
    bias_total = (b.astype(np.float64) * s + beta.astype(np.float64)
                  - mean.astype(np.float64) * s)
    lhsT = np.zeros((4, 128, COUT), dtype=np.float64)
    for di in range(2):
        for dj in range(2):
            wq = (coef[(di, dj)] * s[:, None]).T   # [c, o]
            lhsT[di * 2 + dj, 0:C, :] = wq
            lhsT[di * 2 + dj, C:2 * C, :] = wq
    return (lhsT.astype(ml_dtypes.bfloat16),
            bias_total.astype(np.float32).reshape(COUT, 1))


def build_nc(b_local=B_LOCAL, run_bacc_compile=True):
    nc = bacc.Bacc(None)
    # x pre-tiled on host: [bi, tb, hh, c, hl*w] (fully contiguous tiles)
    x = nc.dram_tensor("x", [b_local, TB, 2, C, HL * W_IMG], BF16,
                       kind="ExternalInput")
    w_lhsT = nc.dram_tensor("w_lhsT", [4, 128, COUT], BF16,
                            kind="ExternalInput")
    bias = nc.dram_tensor("bias", [COUT, 1], F32, kind="ExternalInput")
    # z tiled: [bi, hb, o, 16 rows * WO]; un-tiled on host
    z = nc.dram_tensor("z", [b_local, HB, COUT, 16 * WO], BF16,
                       kind="ExternalOutput")

    with TileContext(nc) as tc:
        with (
            tc.tile_pool(name="consts", bufs=1) as cpool,
            tc.tile_pool(name="xin", bufs=3) as xpool,
            tc.tile_pool(name="psum", bufs=2, space="PSUM") as ppool,
            tc.tile_pool(name="zout", bufs=4) as zpool,
        ):
            w_sb = []
            for q in range(4):
                wt = cpool.tile([128, COUT], BF16, name=f"w{q}_sb")
                nc.sync.dma_start(out=wt[:], in_=w_lhsT[q])
                w_sb.append(wt)
            bias_sb = cpool.tile([COUT, 1], F32)
            nc.sync.dma_start(out=bias_sb[:], in_=bias[:])

            for bi in range(b_local):
                for tb in range(TB):
                    # 128 input rows -> 64 output rows per block.
                    # partition = (hh, c); per partition: 64 rows x 256 w
                    # bf16 = 32 KiB fully contiguous in DRAM.  One DMA per
                    # hh half: DRAM outer dim = c (64) -> descriptors
                    # spray across all 16 SDMA engines; h-half matmuls
                    # start as soon as their half has landed.
                    xt = xpool.tile([128, HL * W_IMG], BF16)
                    for hh in range(2):
                        nc.sync.dma_start(
                            out=xt[C * hh:C * (hh + 1), :],
                            in_=x[bi, tb, hh],
                        )
                    # free f = hl*256 + w = il*512 + di*256 + j*2 + dj
                    xv = xt.rearrange(
                        "p (il di j dj) -> p di dj il j", di=2, j=WO, dj=2
                    )
                    for h in range(2):      # partition half (64 input rows)
                        for pt in range(2):  # 16-output-row psum tile
                            ps = ppool.tile([COUT, 2048], F32)
                            # region gg*512 <- output rows
                            # 64tb + 32h + 16pt + (4gg + 0..3)
                            for q in range(4):
                                di, dj = q // 2, q % 2
                                lw = w_sb[q][C * h:C * (h + 1), :]
                                for gg in range(4):
                                    il0 = 16 * pt + 4 * gg
                                    nc.tensor.matmul(
                                        ps[:, gg * 512:gg * 512 + 512],
                                        lhsT=lw,
                                        rhs=xv[C * h:C * (h + 1), di, dj,
                                               il0:il0 + 4, :],
                                        start=(q == 0),
                                        stop=(q == 3),
                                    )
                            zt = zpool.tile([COUT, 2048], BF16)
                            # bias + ReLU in one DVE pass: max(ps+bias, 0)
                            nc.vector.tensor_scalar(
                                zt[:], ps[:], bias_sb[:, 0:1], 0.0,
                                mybir.AluOpType.add, mybir.AluOpType.max,
                            )
                            hb = 4 * tb + 2 * h + pt
                            # contiguous 512 KiB store, 4 KiB descriptors,
                            # DRAM outer dim o=128 -> sprays all engines.
                            # ACT ring: never blocks loads on the SP ring.
                            nc.scalar.dma_start(
                                out=z[bi, hb], in_=zt[:],
                            )
    if run_bacc_compile:
        nc.compile()
    return nc


_NC_CACHE = {}


def _get_nc():
    if "nc" not in _NC_CACHE:
        _NC_CACHE["nc"] = build_nc()
    return _NC_CACHE["nc"]


def kernel(x, W, b, gamma, beta, mean, var, _trace=False):
    x = np.asarray(x, dtype=np.float32)
    lhsT, bias_col = _fold_weights(
        np.asarray(W), np.asarray(b), np.asarray(gamma),
        np.asarray(beta), np.asarray(mean), np.asarray(var),
    )

    # Pre-tile to [b, tb, hh, c, hl, w] bf16 (contiguous per (b,tb,hh))
    x_bf = x.astype(ml_dtypes.bfloat16)
    x_t = np.ascontiguousarray(
        x_bf.reshape(B, C, TB, 2, HL, W_IMG).transpose(0, 2, 3, 1, 4, 5)
    ).reshape(B, TB, 2, C, HL * W_IMG)

    nc = _get_nc()
    in_maps = []
    for core in range(N_CORES):
        xs = np.ascontiguousarray(x_t[core * B_LOCAL:(core + 1) * B_LOCAL])
        in_maps.append({"x": xs, "w_lhsT": lhsT, "bias": bias_col})

    res = run_bass_kernel_spmd(
        nc, in_maps, list(range(N_CORES)), trace=_trace
    )
    # z tiles [b_local, hb, o, 16*WO] bf16 -> [B, COUT, HO, WO] f32
    zt = np.concatenate(
        [np.asarray(res.results[i]["z"]) for i in range(N_CORES)], axis=0
    ).astype(np.float32)
    out = np.ascontiguousarray(
        zt.reshape(B, HB, COUT, 16, WO).transpose(0, 2, 1, 3, 4)
    ).reshape(B, COUT, HO, WO)
    if _trace:
        return out, res
    return out
